# revision 4
# baseline (speedup 1.0000x reference)
"""Trainium2 Bass kernel for nn_AttentionHead (B=8, T=4096, D=512, d_k=d_v=64).

Strategy: pure data parallelism — one batch element per NeuronCore (8 cores).
Per core:
  QT[64,T]  = Wq^T @ q^T          (contraction over d_model, PSUM-accumulated)
  KT[64,TK] = Wk^T @ k^T
  V  [TK,65]= [v @ Wv * m, m]     (mask folded in: masked key rows zeroed;
                                   col 64 = 0/1 mask -> softmax denominator)
  ST tiles [128j, 512i] = K Q^T   (matmul, contraction 64)
  PT = exp(ST / sqrt(512))        (ScalarE, batched over PSUM groups;
                                   no max-subtraction: |scores| <~ 5)
  OTe[65, 512i] += V_ext^T @ PT   (contraction over keys; row 64 = row sum)
  O = OTe[:64] * (1/OTe[64])      (reciprocal + ones-matmul broadcast)
Host transposes q/k/v shards in, output [64,T] back out.
"""

import sys

import numpy as np

sys.path.insert(0, "/opt/trn_rl_repo")

import concourse.bass as bass  # noqa: F401  (engine namespaces live on nc)
import concourse.mybir as mybir
import concourse.tile as tile
from concourse import bacc
from concourse.bass_utils import run_bass_kernel_spmd

B, T, D, DK = 8, 4096, 512, 64
N_CORES = 8
F32 = mybir.dt.float32
F32R = mybir.dt.float32r
EXP = mybir.ActivationFunctionType.Exp
SCALE = 1.0 / float(np.sqrt(512.0))

_NC_CACHE: dict[int, object] = {}


def _build(tk: int):
    """Build + compile the per-core graph for TK key positions."""
    nj = tk // 128   # 128-row key chunks
    ntb = tk // 512  # 512-col blocks of kT/vT
    nib = T // 512   # query i-blocks

    nc = bacc.Bacc(None, target_bir_lowering=False)

    qT = nc.declare_dram_parameter("qT", [D, T], F32R, isOutput=False)
    kT = nc.declare_dram_parameter("kT", [D, tk], F32R, isOutput=False)
    vT = nc.declare_dram_parameter("vT", [D, tk], F32R, isOutput=False)
    wq = nc.declare_dram_parameter("wq", [4, 128, DK], F32R, isOutput=False)
    wk = nc.declare_dram_parameter("wk", [4, 128, DK], F32R, isOutput=False)
    wv = nc.declare_dram_parameter("wv", [4, 128, DK], F32R, isOutput=False)
    m01 = nc.declare_dram_parameter("m01", [128, nj], F32, isOutput=False)
    out = nc.declare_dram_parameter("out", [DK, T], F32, isOutput=True)

    # j-chunk groups sharing one PSUM region (3 chunks = 3 banks)
    groups = [list(range(g, min(g + 3, nj))) for g in range(0, nj, 3)]

    with tile.TileContext(nc) as tc:
        with tc.tile_pool(name="const", bufs=1) as constp:
            wq_sb = constp.tile([128, 4, DK], F32R, tag="wq")
            wk_sb = constp.tile([128, 4, DK], F32R, tag="wk")
            wv_sb = constp.tile([128, 4, DK], F32R, tag="wv")
            m01_sb = constp.tile([128, nj], F32, tag="m01")
            ones64 = constp.tile([1, DK], F32R, tag="ones")
            QT = constp.tile([DK, T], F32R, tag="QT")
            KT = constp.tile([DK, tk], F32R, tag="KT")
            VE = constp.tile([128, nj, DK + 1], F32R, tag="VE")

            for c in range(4):
                nc.sync.dma_start(wq_sb[:, c, :], wq[c, :, :])
                nc.sync.dma_start(wk_sb[:, c, :], wk[c, :, :])
                nc.sync.dma_start(wv_sb[:, c, :], wv[c, :, :])
            nc.sync.dma_start(m01_sb[:], m01[:, :])
            ones_f32 = constp.tile([1, DK], F32, tag="ones_f32")
            nc.vector.memset(ones_f32[:], 1.0)
            nc.vector.tensor_copy(ones64[:], ones_f32[:])

            # ---------------- phase 1: projections ----------------
            with (
                tc.tile_pool(name="stage", bufs=12) as stage,
                tc.tile_pool(name="psproj", bufs=4, space="PSUM") as psproj,
            ):
                # K^T and V (natural) from k/v shards
                for t in range(ntb):
                    kch = [stage.tile([128, 512], F32R, tag="stg", name=f"kch{t}_{c}") for c in range(4)]
                    for c in range(4):
                        nc.sync.dma_start(
                            kch[c][:], kT[c * 128:(c + 1) * 128, t * 512:(t + 1) * 512]
                        )
                    ps = psproj.tile([DK, 512], F32, tag="pp")
                    for c in range(4):
                        nc.tensor.matmul(
                            ps[:],
                            wk_sb[:, c, :],
                            kch[c][:],
                            start=(c == 0),
                            stop=(c == 3),
                        )
                    nc.vector.tensor_copy(KT[:, t * 512:(t + 1) * 512], ps[:])

                    vch = [stage.tile([128, 512], F32R, tag="stg", name=f"vch{t}_{c}") for c in range(4)]
                    for c in range(4):
                        nc.sync.dma_start(
                            vch[c][:], vT[c * 128:(c + 1) * 128, t * 512:(t + 1) * 512]
                        )
                    for j in range(4):
                        jt = t * 4 + j
                        psv = psproj.tile([128, DK], F32, tag="pp")
                        for c in range(4):
                            nc.tensor.matmul(
                                psv[:],
                                vch[c][:, j * 128:(j + 1) * 128],
                                wv_sb[:, c, :],
                                start=(c == 0),
                                stop=(c == 3),
                            )
                        # zero masked key rows; col 64 = 0/1 mask (denominator)
                        nc.vector.tensor_scalar_mul(
                            VE[:, jt, 0:DK], psv[:], m01_sb[:, jt:jt + 1]
                        )
                        nc.vector.tensor_copy(
                            VE[:, jt, DK:DK + 1], m01_sb[:, jt:jt + 1]
                        )

                # Q^T
                for t in range(T // 512):
                    qch = [stage.tile([128, 512], F32R, tag="stg", name=f"qch{t}_{c}") for c in range(4)]
                    for c in range(4):
                        nc.sync.dma_start(
                            qch[c][:], qT[c * 128:(c + 1) * 128, t * 512:(t + 1) * 512]
                        )
                    ps = psproj.tile([DK, 512], F32, tag="pp")
                    for c in range(4):
                        nc.tensor.matmul(
                            ps[:],
                            wq_sb[:, c, :],
                            qch[c][:],
                            start=(c == 0),
                            stop=(c == 3),
                        )
                    nc.vector.tensor_copy(QT[:, t * 512:(t + 1) * 512], ps[:])

            # ---------------- phase 2: attention ----------------
            with (
                tc.tile_pool(name="pt", bufs=2) as ptp,
                tc.tile_pool(name="outp", bufs=2) as outp,
                tc.tile_pool(name="ps_st", bufs=2, space="PSUM") as ps_st,
                tc.tile_pool(name="ps_om", bufs=2, space="PSUM") as ps_om,
            ):
                for ib in range(nib):
                    isl = slice(ib * 512, (ib + 1) * 512)
                    ot = ps_om.tile([DK + 1, 512], F32, tag="om")
                    n_mm = 0
                    for chunks in groups:
                        w = 512 * len(chunks)
                        st = ps_st.tile([128, w], F32, tag="st")
                        for ci, jc in enumerate(chunks):
                            nc.tensor.matmul(
                                st[:, ci * 512:(ci + 1) * 512],
                                KT[:, jc * 128:(jc + 1) * 128],
                                QT[:, isl],
                                start=True,
                                stop=True,
                            )
                        pt = ptp.tile([128, w], F32R, tag="pt")
                        nc.scalar.activation(pt[:], st[:], EXP, scale=SCALE)
                        for ci, jc in enumerate(chunks):
                            nc.tensor.matmul(
                                ot[:],
                                VE[:, jc, :],
                                pt[:, ci * 512:(ci + 1) * 512],
                                start=(n_mm == 0),
                                stop=(n_mm == nj - 1),
                                skip_group_check=True,
                            )
                            n_mm += 1
                    recip = outp.tile([1, 512], F32R, tag="rc")
                    with nc.allow_low_precision("f32r broadcast rhs"):
                        nc.vector.reciprocal(recip[:], ot[DK:DK + 1, :])
                    bc = ps_om.tile([DK, 512], F32, tag="om")
                    nc.tensor.matmul(
                        bc[:],
                        ones64[:],
                        recip[:],
                        start=True,
                        stop=True,
                    )
                    bcs = outp.tile([DK, 512], F32, tag="bc")
                    nc.vector.tensor_copy(bcs[:], bc[:])
                    o = outp.tile([DK, 512], F32, tag="o")
                    nc.vector.tensor_mul(o[:], ot[0:DK, :], bcs[:])
                    nc.sync.dma_start(out[:, isl], o[:])

    nc.compile()
    return nc


def _get_nc(tk: int):
    if tk not in _NC_CACHE:
        _NC_CACHE[tk] = _build(tk)
    return _NC_CACHE[tk]


def _prep_in_maps(k, v, q, pad_mask, Wk, Wq, Wv, tk: int):
    wq_r = np.ascontiguousarray(Wq.reshape(4, 128, DK), dtype=np.float32)
    wk_r = np.ascontiguousarray(Wk.reshape(4, 128, DK), dtype=np.float32)
    wv_r = np.ascontiguousarray(Wv.reshape(4, 128, DK), dtype=np.float32)
    in_maps = []
    for b in range(B):
        m = (pad_mask[b, 0] != 1).astype(np.float32)  # 1.0 keep, 0.0 masked
        in_maps.append(
            {
                "qT": np.ascontiguousarray(q[b].T, dtype=np.float32),
                "kT": np.ascontiguousarray(k[b].T, dtype=np.float32),
                "vT": np.ascontiguousarray(v[b].T, dtype=np.float32),
                "wq": wq_r,
                "wk": wk_r,
                "wv": wv_r,
                "m01": np.ascontiguousarray(m.reshape(tk // 128, 128).T),
            }
        )
    return in_maps


def _run(k, v, q, pad_mask, Wk, Wq, Wv, trace=False, **spmd_kwargs):
    tk = T
    nc = _get_nc(tk)
    in_maps = _prep_in_maps(k, v, q, pad_mask, Wk, Wq, Wv, tk)
    res = run_bass_kernel_spmd(
        nc, in_maps, core_ids=list(range(N_CORES)), trace=trace, **spmd_kwargs
    )
    outs = np.stack(
        [np.asarray(res.results[b]["out"]).T for b in range(B)], axis=0
    )
    return outs.astype(np.float32), res


def kernel(k, v, q, pad_mask, Wk, Wq, Wv):
    outs, _ = _run(k, v, q, pad_mask, Wk, Wq, Wv, trace=False)
    return outs


# revision 6
# speedup vs baseline: 1.5847x; 1.5847x over previous
"""Trainium2 Bass kernel for nn_AttentionHead (B=8, T=4096, D=512, d_k=d_v=64).

Strategy: pure data parallelism — one batch element per NeuronCore (8 cores).
Per core:
  QT[64,T]  = Wq^T @ q^T          (contraction over d_model, PSUM-accumulated)
  KT[64,TK] = Wk^T @ k^T
  V  [TK,65]= [v @ Wv * m, m]     (mask folded in: masked key rows zeroed;
                                   col 64 = 0/1 mask -> softmax denominator)
  ST tiles [128j, 512i] = K Q^T   (matmul, contraction 64)
  PT = exp(ST / sqrt(512))        (ScalarE, batched over PSUM groups;
                                   no max-subtraction: |scores| <~ 5)
  OTe[65, 512i] += V_ext^T @ PT   (contraction over keys; row 64 = row sum)
  O = OTe[:64] * (1/OTe[64])      (reciprocal + ones-matmul broadcast)
Host transposes q/k/v shards in, output [64,T] back out.
"""

import sys

import numpy as np

sys.path.insert(0, "/opt/trn_rl_repo")

import concourse.bass as bass  # noqa: F401  (engine namespaces live on nc)
import concourse.mybir as mybir
import concourse.tile as tile
from concourse import bacc
from concourse.bass_utils import run_bass_kernel_spmd

B, T, D, DK = 8, 4096, 512, 64
N_CORES = 8
F32 = mybir.dt.float32
F32R = mybir.dt.float32r
BF16 = mybir.dt.bfloat16
EXP = mybir.ActivationFunctionType.Exp
SCALE = 1.0 / float(np.sqrt(512.0))

_NC_CACHE: dict[int, object] = {}


def _build(tk: int):
    """Build + compile the per-core graph for TK key positions."""
    nj = tk // 128   # 128-row key chunks
    ntb = tk // 512  # 512-col blocks of kT/vT
    nib = T // 512   # query i-blocks

    nc = bacc.Bacc(None, target_bir_lowering=False)

    qT = nc.declare_dram_parameter("qT", [D, T], BF16, isOutput=False)
    kT = nc.declare_dram_parameter("kT", [D, tk], BF16, isOutput=False)
    vT = nc.declare_dram_parameter("vT", [D, tk], BF16, isOutput=False)
    wq = nc.declare_dram_parameter("wq", [4, 128, DK], BF16, isOutput=False)
    wk = nc.declare_dram_parameter("wk", [4, 128, DK], BF16, isOutput=False)
    wv = nc.declare_dram_parameter("wv", [4, 128, DK], BF16, isOutput=False)
    m01 = nc.declare_dram_parameter("m01", [128, nj], F32, isOutput=False)
    out = nc.declare_dram_parameter("out", [DK, T], F32, isOutput=True)

    # j-chunk groups sharing one PSUM region (3 chunks = 3 banks)
    groups = [list(range(g, min(g + 3, nj))) for g in range(0, nj, 3)]

    with tile.TileContext(nc) as tc:
        with tc.tile_pool(name="const", bufs=1) as constp:
            wq_sb = constp.tile([128, 4, DK], BF16, tag="wq")
            wk_sb = constp.tile([128, 4, DK], BF16, tag="wk")
            wv_sb = constp.tile([128, 4, DK], BF16, tag="wv")
            m01_sb = constp.tile([128, nj], F32, tag="m01")
            ones64 = constp.tile([1, DK], BF16, tag="ones")
            QT = constp.tile([DK, T], BF16, tag="QT")
            KT = constp.tile([DK, tk], BF16, tag="KT")
            VE = constp.tile([128, nj, DK + 1], BF16, tag="VE")

            for c in range(4):
                nc.sync.dma_start(wq_sb[:, c, :], wq[c, :, :])
                nc.sync.dma_start(wk_sb[:, c, :], wk[c, :, :])
                nc.sync.dma_start(wv_sb[:, c, :], wv[c, :, :])
            nc.sync.dma_start(m01_sb[:], m01[:, :])
            ones_f32 = constp.tile([1, DK], F32, tag="ones_f32")
            nc.vector.memset(ones_f32[:], 1.0)
            nc.vector.tensor_copy(ones64[:], ones_f32[:])

            # ---------------- phase 1: projections ----------------
            with (
                tc.tile_pool(name="stage", bufs=12) as stage,
                tc.tile_pool(name="psproj", bufs=4, space="PSUM") as psproj,
            ):
                # K^T and V (natural) from k/v shards
                for t in range(ntb):
                    kch = [stage.tile([128, 512], BF16, tag="stg", name=f"kch{t}_{c}") for c in range(4)]
                    for c in range(4):
                        nc.sync.dma_start(
                            kch[c][:], kT[c * 128:(c + 1) * 128, t * 512:(t + 1) * 512]
                        )
                    ps = psproj.tile([DK, 512], F32, tag="pp")
                    for c in range(4):
                        nc.tensor.matmul(
                            ps[:],
                            wk_sb[:, c, :],
                            kch[c][:],
                            start=(c == 0),
                            stop=(c == 3),
                        )
                    nc.vector.tensor_copy(KT[:, t * 512:(t + 1) * 512], ps[:])

                    vch = [stage.tile([128, 512], BF16, tag="stg", name=f"vch{t}_{c}") for c in range(4)]
                    for c in range(4):
                        nc.sync.dma_start(
                            vch[c][:], vT[c * 128:(c + 1) * 128, t * 512:(t + 1) * 512]
                        )
                    for j in range(4):
                        jt = t * 4 + j
                        psv = psproj.tile([128, DK], F32, tag="pp")
                        for c in range(4):
                            nc.tensor.matmul(
                                psv[:],
                                vch[c][:, j * 128:(j + 1) * 128],
                                wv_sb[:, c, :],
                                start=(c == 0),
                                stop=(c == 3),
                            )
                        # zero masked key rows; col 64 = 0/1 mask (denominator)
                        nc.vector.tensor_scalar_mul(
                            VE[:, jt, 0:DK], psv[:], m01_sb[:, jt:jt + 1]
                        )
                        nc.vector.tensor_copy(
                            VE[:, jt, DK:DK + 1], m01_sb[:, jt:jt + 1]
                        )

                # Q^T
                for t in range(T // 512):
                    qch = [stage.tile([128, 512], BF16, tag="stg", name=f"qch{t}_{c}") for c in range(4)]
                    for c in range(4):
                        nc.sync.dma_start(
                            qch[c][:], qT[c * 128:(c + 1) * 128, t * 512:(t + 1) * 512]
                        )
                    ps = psproj.tile([DK, 512], F32, tag="pp")
                    for c in range(4):
                        nc.tensor.matmul(
                            ps[:],
                            wq_sb[:, c, :],
                            qch[c][:],
                            start=(c == 0),
                            stop=(c == 3),
                        )
                    nc.vector.tensor_copy(QT[:, t * 512:(t + 1) * 512], ps[:])

            # ---------------- phase 2: attention ----------------
            with (
                tc.tile_pool(name="pt", bufs=2) as ptp,
                tc.tile_pool(name="outp", bufs=2) as outp,
                tc.tile_pool(name="ps_st", bufs=2, space="PSUM") as ps_st,
                tc.tile_pool(name="ps_om", bufs=2, space="PSUM") as ps_om,
            ):
                for ib in range(nib):
                    isl = slice(ib * 512, (ib + 1) * 512)
                    ot = ps_om.tile([DK + 1, 512], F32, tag="om")
                    n_mm = 0
                    for chunks in groups:
                        w = 512 * len(chunks)
                        st = ps_st.tile([128, w], F32, tag="st")
                        for ci, jc in enumerate(chunks):
                            nc.tensor.matmul(
                                st[:, ci * 512:(ci + 1) * 512],
                                KT[:, jc * 128:(jc + 1) * 128],
                                QT[:, isl],
                                start=True,
                                stop=True,
                            )
                        pt = ptp.tile([128, w], BF16, tag="pt")
                        nc.scalar.activation(pt[:], st[:], EXP, scale=SCALE)
                        for ci, jc in enumerate(chunks):
                            nc.tensor.matmul(
                                ot[:],
                                VE[:, jc, :],
                                pt[:, ci * 512:(ci + 1) * 512],
                                start=(n_mm == 0),
                                stop=(n_mm == nj - 1),
                                skip_group_check=True,
                            )
                            n_mm += 1
                    recip = outp.tile([1, 512], BF16, tag="rc")
                    with nc.allow_low_precision("bf16 broadcast rhs"):
                        nc.vector.reciprocal(recip[:], ot[DK:DK + 1, :])
                    bc = ps_om.tile([DK, 512], F32, tag="om")
                    nc.tensor.matmul(
                        bc[:],
                        ones64[:],
                        recip[:],
                        start=True,
                        stop=True,
                    )
                    bcs = outp.tile([DK, 512], F32, tag="bc")
                    nc.vector.tensor_copy(bcs[:], bc[:])
                    o = outp.tile([DK, 512], F32, tag="o")
                    nc.vector.tensor_mul(o[:], ot[0:DK, :], bcs[:])
                    nc.sync.dma_start(out[:, isl], o[:])

    nc.compile()
    return nc


def _get_nc(tk: int):
    if tk not in _NC_CACHE:
        _NC_CACHE[tk] = _build(tk)
    return _NC_CACHE[tk]


def _prep_in_maps(k, v, q, pad_mask, Wk, Wq, Wv, tk: int, keep_idx):
    """Per-core shard prep. Keys are compacted to the unmasked positions
    (masked keys contribute exactly 0 to softmax numerator and denominator),
    zero-padded up to tk; m01 marks live rows."""
    import ml_dtypes

    bf16 = ml_dtypes.bfloat16
    wq_r = np.ascontiguousarray(Wq.reshape(4, 128, DK)).astype(bf16)
    wk_r = np.ascontiguousarray(Wk.reshape(4, 128, DK)).astype(bf16)
    wv_r = np.ascontiguousarray(Wv.reshape(4, 128, DK)).astype(bf16)
    in_maps = []
    for b in range(B):
        idx = keep_idx[b]
        n = len(idx)
        kc = np.zeros((tk, D), np.float32)
        vc = np.zeros((tk, D), np.float32)
        kc[:n] = k[b][idx]
        vc[:n] = v[b][idx]
        m = np.zeros(tk, np.float32)
        m[:n] = 1.0
        in_maps.append(
            {
                "qT": np.ascontiguousarray(q[b].T).astype(bf16),
                "kT": np.ascontiguousarray(kc.T).astype(bf16),
                "vT": np.ascontiguousarray(vc.T).astype(bf16),
                "wq": wq_r,
                "wk": wk_r,
                "wv": wv_r,
                "m01": np.ascontiguousarray(m.reshape(tk // 128, 128).T),
            }
        )
    return in_maps


def _run(k, v, q, pad_mask, Wk, Wq, Wv, trace=False, **spmd_kwargs):
    keep_idx = [np.flatnonzero(pad_mask[b, 0] != 1) for b in range(B)]
    max_keep = max(len(i) for i in keep_idx)
    tk = max(512, -(-max_keep // 512) * 512)  # round up to 512-multiple
    nc = _get_nc(tk)
    in_maps = _prep_in_maps(k, v, q, pad_mask, Wk, Wq, Wv, tk, keep_idx)
    res = run_bass_kernel_spmd(
        nc, in_maps, core_ids=list(range(N_CORES)), trace=trace, **spmd_kwargs
    )
    outs = np.stack(
        [np.asarray(res.results[b]["out"]).T for b in range(B)], axis=0
    )
    return outs.astype(np.float32), res


def kernel(k, v, q, pad_mask, Wk, Wq, Wv):
    outs, _ = _run(k, v, q, pad_mask, Wk, Wq, Wv, trace=False)
    return outs


# revision 7
# speedup vs baseline: 1.5923x; 1.0048x over previous
"""Trainium2 Bass kernel for nn_AttentionHead (B=8, T=4096, D=512, d_k=d_v=64).

Strategy: pure data parallelism — one batch element per NeuronCore (8 cores).
Per core:
  QT[64,T]  = Wq^T @ q^T          (contraction over d_model, PSUM-accumulated)
  KT[64,TK] = Wk^T @ k^T
  V  [TK,65]= [v @ Wv * m, m]     (mask folded in: masked key rows zeroed;
                                   col 64 = 0/1 mask -> softmax denominator)
  ST tiles [128j, 512i] = K Q^T   (matmul, contraction 64)
  PT = exp(ST / sqrt(512))        (ScalarE, batched over PSUM groups;
                                   no max-subtraction: |scores| <~ 5)
  OTe[65, 512i] += V_ext^T @ PT   (contraction over keys; row 64 = row sum)
  O = OTe[:64] * (1/OTe[64])      (reciprocal + ones-matmul broadcast)
Host transposes q/k/v shards in, output [64,T] back out.
"""

import sys

import numpy as np

sys.path.insert(0, "/opt/trn_rl_repo")

import concourse.bass as bass  # noqa: F401  (engine namespaces live on nc)
import concourse.mybir as mybir
import concourse.tile as tile
from concourse import bacc
from concourse.bass_utils import run_bass_kernel_spmd

B, T, D, DK = 8, 4096, 512, 64
N_CORES = 8
F32 = mybir.dt.float32
F32R = mybir.dt.float32r
BF16 = mybir.dt.bfloat16
EXP = mybir.ActivationFunctionType.Exp
SCALE = 1.0 / float(np.sqrt(512.0))

_NC_CACHE: dict[int, object] = {}


def _build(tk: int):
    """Build + compile the per-core graph for TK key positions."""
    nj = tk // 128   # 128-row key chunks
    ntb = tk // 512  # 512-col blocks of kT/vT
    nib = T // 512   # query i-blocks

    nc = bacc.Bacc(None, target_bir_lowering=False)

    qT = nc.declare_dram_parameter("qT", [D, T], BF16, isOutput=False)
    kT = nc.declare_dram_parameter("kT", [D, tk], BF16, isOutput=False)
    vT = nc.declare_dram_parameter("vT", [D, tk], BF16, isOutput=False)
    wq = nc.declare_dram_parameter("wq", [4, 128, DK], BF16, isOutput=False)
    wk = nc.declare_dram_parameter("wk", [4, 128, DK], BF16, isOutput=False)
    wv = nc.declare_dram_parameter("wv", [4, 128, DK], BF16, isOutput=False)
    m01 = nc.declare_dram_parameter("m01", [128, nj], F32, isOutput=False)
    out = nc.declare_dram_parameter("out", [DK, T], F32, isOutput=True)

    # j-chunk groups sharing one PSUM region (3 chunks = 3 banks)
    groups = [list(range(g, min(g + 3, nj))) for g in range(0, nj, 3)]

    with tile.TileContext(nc) as tc:
        with tc.tile_pool(name="const", bufs=1) as constp:
            wq_sb = constp.tile([128, 4, DK], BF16, tag="wq")
            wk_sb = constp.tile([128, 4, DK], BF16, tag="wk")
            wv_sb = constp.tile([128, 4, DK], BF16, tag="wv")
            m01_sb = constp.tile([128, nj], F32, tag="m01")
            ones64 = constp.tile([1, DK], BF16, tag="ones")
            QT = constp.tile([DK, T], BF16, tag="QT")
            KT = constp.tile([DK, tk], BF16, tag="KT")
            VE = constp.tile([128, nj, DK + 1], BF16, tag="VE")

            for c in range(4):
                nc.sync.dma_start(wq_sb[:, c, :], wq[c, :, :])
                nc.sync.dma_start(wk_sb[:, c, :], wk[c, :, :])
                nc.sync.dma_start(wv_sb[:, c, :], wv[c, :, :])
            nc.sync.dma_start(m01_sb[:], m01[:, :])
            ones_f32 = constp.tile([1, DK], F32, tag="ones_f32")
            nc.vector.memset(ones_f32[:], 1.0)
            nc.vector.tensor_copy(ones64[:], ones_f32[:])

            # ---------------- phase 1: projections ----------------
            with (
                tc.tile_pool(name="stage", bufs=12) as stage,
                tc.tile_pool(name="psproj", bufs=4, space="PSUM") as psproj,
            ):
                # K^T and V (natural) from k/v shards
                for t in range(ntb):
                    kch = [stage.tile([128, 512], BF16, tag="stg", name=f"kch{t}_{c}") for c in range(4)]
                    for c in range(4):
                        nc.sync.dma_start(
                            kch[c][:], kT[c * 128:(c + 1) * 128, t * 512:(t + 1) * 512]
                        )
                    ps = psproj.tile([DK, 512], F32, tag="pp")
                    for c in range(4):
                        nc.tensor.matmul(
                            ps[:],
                            wk_sb[:, c, :],
                            kch[c][:],
                            start=(c == 0),
                            stop=(c == 3),
                        )
                    nc.vector.tensor_copy(KT[:, t * 512:(t + 1) * 512], ps[:])

                    vch = [stage.tile([128, 512], BF16, tag="stg", name=f"vch{t}_{c}") for c in range(4)]
                    for c in range(4):
                        nc.sync.dma_start(
                            vch[c][:], vT[c * 128:(c + 1) * 128, t * 512:(t + 1) * 512]
                        )
                    for j in range(4):
                        jt = t * 4 + j
                        psv = psproj.tile([128, DK], F32, tag="pp")
                        for c in range(4):
                            nc.tensor.matmul(
                                psv[:],
                                vch[c][:, j * 128:(j + 1) * 128],
                                wv_sb[:, c, :],
                                start=(c == 0),
                                stop=(c == 3),
                            )
                        # zero masked key rows; col 64 = 0/1 mask (denominator)
                        nc.vector.tensor_scalar_mul(
                            VE[:, jt, 0:DK], psv[:], m01_sb[:, jt:jt + 1]
                        )
                        nc.vector.tensor_copy(
                            VE[:, jt, DK:DK + 1], m01_sb[:, jt:jt + 1]
                        )

                # Q^T
                for t in range(T // 512):
                    qch = [stage.tile([128, 512], BF16, tag="stg", name=f"qch{t}_{c}") for c in range(4)]
                    for c in range(4):
                        nc.sync.dma_start(
                            qch[c][:], qT[c * 128:(c + 1) * 128, t * 512:(t + 1) * 512]
                        )
                    ps = psproj.tile([DK, 512], F32, tag="pp")
                    for c in range(4):
                        nc.tensor.matmul(
                            ps[:],
                            wq_sb[:, c, :],
                            qch[c][:],
                            start=(c == 0),
                            stop=(c == 3),
                        )
                    nc.vector.tensor_copy(QT[:, t * 512:(t + 1) * 512], ps[:])

            # ---------------- phase 2: attention ----------------
            # i-blocks processed in pairs with lag-2 software pipelining on
            # the PE stream (ST(k) -> exp(k-1) -> OT(k-2)) so the PE never
            # micro-idles waiting on ScalarE (HAM stays warm).
            o_un = constp.tile([DK, T], F32, tag="o_un")
            recip_all = constp.tile([1, T], BF16, tag="recip_all")
            with (
                tc.tile_pool(name="pt", bufs=3) as ptp,
                tc.tile_pool(name="outp", bufs=2) as outp,
                tc.tile_pool(name="ps_st", bufs=2, space="PSUM") as ps_st,
                tc.tile_pool(name="ps_om", bufs=2, space="PSUM") as ps_om,
            ):
                for pair in range(nib // 2):
                    blocks = (2 * pair, 2 * pair + 1)
                    ot = {
                        ib: ps_om.tile([DK + 1, 512], F32, tag="om", name=f"ot{ib}")
                        for ib in blocks
                    }
                    n_mm = {ib: 0 for ib in blocks}
                    seq = [(g, ib) for g in range(len(groups)) for ib in blocks]
                    stash = {}

                    def emit_st(g, ib):
                        chunks = groups[g]
                        w = 512 * len(chunks)
                        st = ps_st.tile([128, w], F32, tag="st", name=f"st{g}_{ib}")
                        for ci, jc in enumerate(chunks):
                            nc.tensor.matmul(
                                st[:, ci * 512:(ci + 1) * 512],
                                KT[:, jc * 128:(jc + 1) * 128],
                                QT[:, ib * 512:(ib + 1) * 512],
                                start=True,
                                stop=True,
                            )
                        stash[(g, ib)] = st

                    def emit_exp(g, ib):
                        st = stash[(g, ib)]
                        w = st.shape[-1]
                        pt = ptp.tile([128, w], BF16, tag="pt", name=f"pt{g}_{ib}")
                        nc.scalar.activation(pt[:], st[:], EXP, scale=SCALE)
                        stash[(g, ib)] = pt

                    def emit_ot(g, ib):
                        pt = stash.pop((g, ib))
                        for ci, jc in enumerate(groups[g]):
                            nc.tensor.matmul(
                                ot[ib][:],
                                VE[:, jc, :],
                                pt[:, ci * 512:(ci + 1) * 512],
                                start=(n_mm[ib] == 0),
                                stop=(n_mm[ib] == nj - 1),
                                skip_group_check=True,
                            )
                            n_mm[ib] += 1

                    for k in range(len(seq) + 2):
                        if k < len(seq):
                            emit_st(*seq[k])
                        if 1 <= k <= len(seq):
                            emit_exp(*seq[k - 1])
                        if k >= 2:
                            emit_ot(*seq[k - 2])

                    for ib in blocks:
                        isl = slice(ib * 512, (ib + 1) * 512)
                        nc.vector.tensor_copy(o_un[:, isl], ot[ib][0:DK, :])
                        with nc.allow_low_precision("bf16 broadcast rhs"):
                            nc.vector.reciprocal(
                                recip_all[:, isl], ot[ib][DK:DK + 1, :]
                            )

                # tail: broadcast 1/rowsum across partitions and normalize
                for ib in range(nib):
                    isl = slice(ib * 512, (ib + 1) * 512)
                    bc = ps_om.tile([DK, 512], F32, tag="om", name=f"bc{ib}")
                    nc.tensor.matmul(
                        bc[:], ones64[:], recip_all[:, isl], start=True, stop=True
                    )
                    o = outp.tile([DK, 512], F32, tag="o", name=f"o{ib}")
                    nc.vector.tensor_mul(o[:], o_un[:, isl], bc[:])
                    nc.sync.dma_start(out[:, isl], o[:])

    nc.compile()
    return nc


def _get_nc(tk: int):
    if tk not in _NC_CACHE:
        _NC_CACHE[tk] = _build(tk)
    return _NC_CACHE[tk]


def _prep_in_maps(k, v, q, pad_mask, Wk, Wq, Wv, tk: int, keep_idx):
    """Per-core shard prep. Keys are compacted to the unmasked positions
    (masked keys contribute exactly 0 to softmax numerator and denominator),
    zero-padded up to tk; m01 marks live rows."""
    import ml_dtypes

    bf16 = ml_dtypes.bfloat16
    wq_r = np.ascontiguousarray(Wq.reshape(4, 128, DK)).astype(bf16)
    wk_r = np.ascontiguousarray(Wk.reshape(4, 128, DK)).astype(bf16)
    wv_r = np.ascontiguousarray(Wv.reshape(4, 128, DK)).astype(bf16)
    in_maps = []
    for b in range(B):
        idx = keep_idx[b]
        n = len(idx)
        kc = np.zeros((tk, D), np.float32)
        vc = np.zeros((tk, D), np.float32)
        kc[:n] = k[b][idx]
        vc[:n] = v[b][idx]
        m = np.zeros(tk, np.float32)
        m[:n] = 1.0
        in_maps.append(
            {
                "qT": np.ascontiguousarray(q[b].T).astype(bf16),
                "kT": np.ascontiguousarray(kc.T).astype(bf16),
                "vT": np.ascontiguousarray(vc.T).astype(bf16),
                "wq": wq_r,
                "wk": wk_r,
                "wv": wv_r,
                "m01": np.ascontiguousarray(m.reshape(tk // 128, 128).T),
            }
        )
    return in_maps


def _run(k, v, q, pad_mask, Wk, Wq, Wv, trace=False, **spmd_kwargs):
    keep_idx = [np.flatnonzero(pad_mask[b, 0] != 1) for b in range(B)]
    max_keep = max(len(i) for i in keep_idx)
    tk = max(512, -(-max_keep // 512) * 512)  # round up to 512-multiple
    nc = _get_nc(tk)
    in_maps = _prep_in_maps(k, v, q, pad_mask, Wk, Wq, Wv, tk, keep_idx)
    res = run_bass_kernel_spmd(
        nc, in_maps, core_ids=list(range(N_CORES)), trace=trace, **spmd_kwargs
    )
    outs = np.stack(
        [np.asarray(res.results[b]["out"]).T for b in range(B)], axis=0
    )
    return outs.astype(np.float32), res


def kernel(k, v, q, pad_mask, Wk, Wq, Wv):
    outs, _ = _run(k, v, q, pad_mask, Wk, Wq, Wv, trace=False)
    return outs
